# revision 69
# baseline (speedup 1.0000x reference)
"""Trainium2 Bass kernel for nn_Block_30262339567868 (attention + top-2 MoE block).

Self-contained: takes FULL inputs, shards across 8 NeuronCores internally,
returns the FULL output.

Sharding:
  - Attention: head-parallel (16 heads / 8 cores = 2 heads per core), each core
    produces a partial projection output; host sums partials.
  - MoE: expert-parallel (8 experts / 8 cores), host-side token dispatch
    (gather to per-expert capacity buffers) and gate-weighted scatter-add.
Matmuls run as float32r (tf32-class) except the attention inner (exp(S), V in
bf16). All matmuls use a uniform K=128 contraction (S is zero-padded) — the PE
pays ~200ns per contraction-size switch.
"""

import numpy as np

import concourse.bass as bass
import concourse.mybir as mybir
import concourse.tile as tile
from concourse import bacc
from concourse.bass_utils import run_bass_kernel_spmd
from concourse.masks import make_identity

# Problem shapes (hardcoded per contract)
T = 2048
C = 1024
E = 8
HFF = 4096
NH = 16
HD = 64
NCORES = 8
HPC = NH // NCORES  # heads per core = 2
EPS = 1e-6

F32 = mybir.dt.float32
F32R = mybir.dt.float32r
BF16 = mybir.dt.bfloat16
F8 = mybir.dt.float8e4

# MoE fp8 scale scheme: weights pre-scaled before e4m3 cast; the silu
# input is descaled on-chip (activation scale), the rest is folded into
# the host-side gate weights.
S_W = 32.0   # wg, wd scale
S_U = 16.0   # wu scale == S_H (so h_fp8 = silu(g) * pu directly)
S_H = 16.0

_nc_cache = {}


# --------------------------------------------------------------------------
# Launch A: attention (head-sharded)
# --------------------------------------------------------------------------

def build_attention():
    if "attn" in _nc_cache:
        return _nc_cache["attn"]
    nc = bacc.Bacc("TRN2", target_bir_lowering=False, debug=False,
                   num_devices=NCORES)

    d_xhatT = nc.dram_tensor("xhatT", [C, T], F32R, kind="ExternalInput")
    d_wqkv = nc.dram_tensor("wqkv", [C, 3 * HPC * HD], F32R, kind="ExternalInput")
    d_wproj = nc.dram_tensor("wproj", [HPC * HD, C], F32R, kind="ExternalInput")
    d_ctab = nc.dram_tensor("ctab", [HPC * HD, T], F32, kind="ExternalInput")
    d_stab = nc.dram_tensor("stab", [HPC * HD, T], F32, kind="ExternalInput")
    # additive causal masks for the 4 diagonal k-tiles of a 512-query chunk
    d_mask = nc.dram_tensor("mask", [4, 128, 512], F32R, kind="ExternalInput")
    d_identV = nc.dram_tensor("identV", [128, 128], F32, kind="ExternalInput")
    d_identR = nc.dram_tensor("identR", [128, 128], F32R, kind="ExternalInput")
    d_out = nc.dram_tensor("attn_part", [T, C], F32, kind="ExternalOutput")

    TT = T // 512        # 4 tq chunks
    NTK = T // 128       # 16 tk tiles
    D2 = HPC * HD        # 128
    NKC = C // 128       # 8

    with tile.TileContext(nc) as tc:
        with tc.tile_pool(name="big", bufs=1) as big, \
             tc.tile_pool(name="consts", bufs=1) as consts, \
             tc.tile_pool(name="xstream", bufs=2) as xstream, \
             tc.tile_pool(name="work", bufs=1) as work, \
             tc.tile_pool(name="small", bufs=2) as small, \
             tc.tile_pool(name="ostage", bufs=2) as ostage, \
             tc.tile_pool(name="estrip", bufs=6) as estrip, \
             tc.tile_pool(name="psA", bufs=2, space="PSUM") as psA, \
             tc.tile_pool(name="psS", bufs=2, space="PSUM") as psS, \
             tc.tile_pool(name="psO", bufs=1, space="PSUM") as psO:

            # ---- constants ----
            xhatT_r = d_xhatT.ap().rearrange("(ko p) t -> p ko t", p=128)
            wqkv_r = d_wqkv.ap().rearrange("(ko p) m -> p ko m", p=128)
            wqkv = consts.tile([128, NKC, 3 * D2], F32R)
            wproj = consts.tile([D2, C], F32R)
            ctab = consts.tile([D2, T], F32)
            stab = consts.tile([D2, T], F32)
            maskadd = consts.tile([128, 4, 512], F32R)
            identV = consts.tile([128, 128], F32)
            identR = consts.tile([128, 128], F32R)

            def load_consts():  # queued after chunk 0's x data
                nc.sync.dma_start(ctab[:], d_ctab.ap())
                nc.sync.dma_start(stab[:], d_stab.ap())
                nc.sync.dma_start(identV[:], d_identV.ap())
                nc.sync.dma_start(identR[:], d_identR.ap())
                nc.sync.dma_start(maskadd[:],
                                  d_mask.ap().rearrange("m p f -> p m f"))

            # per-head q/k, real data in rows [0:64], zero-padded [64:128]
            # (uniform K=128 contraction: PE pays ~200ns per size switch)
            qhp = [big.tile([128, T], F32R, name=f"qhp{h}") for h in range(HPC)]
            khp = [big.tile([128, T], F32R, name=f"khp{h}") for h in range(HPC)]
            v2 = big.tile([D2, T], F32)
            vprime = [big.tile([128, NTK, HD + 1], BF16, name=f"vp{h}")
                      for h in range(HPC)]
            yhat = big.tile([D2, T], F32R)
            for h in range(HPC):
                nc.any.memset(vprime[h][:, :, HD:HD + 1], 1.0)
            # zero the pad rows (hidden under the first x-chunk DMA);
            # memset on f32r is invalid ISA -> bounce zeros via an F32 tile
            zsrc = work.tile([HD, T], F32, tag="zsrc")
            nc.gpsimd.memset(zsrc[:], 0.0)
            nc.vector.tensor_copy(qhp[0][HD:, :], zsrc[:])
            nc.vector.tensor_copy(khp[0][HD:, :], zsrc[:])
            nc.scalar.copy(qhp[1][HD:, :], zsrc[:])
            nc.scalar.copy(khp[1][HD:, :], zsrc[:])

            LAG = 3
            pending_proj = []
            # output staged per chunk: [128, 4 t-tiles, C] -> one 2MB DMA
            outT_r = d_out.ap().rearrange("(b tt p) c -> b p tt c",
                                          tt=4, p=128)

            def emit_proj_t(c, tl, obc, dve_only=False):
                # dve_only: inside the AV drain the Scalar engine must finish
                # the trailing exps -- keep proj's bounce copies off it there
                t = 4 * c + tl
                for cc in range(C // 512):
                    pp = psA.tile([128, 512], F32, tag='a')
                    nc.tensor.matmul(pp[:], yhat[:, t * 128:(t + 1) * 128],
                                     wproj[:, cc * 512:(cc + 1) * 512],
                                     start=True, stop=True)
                    if dve_only or (t + cc) % 2 == 0:
                        nc.vector.tensor_copy(
                            obc[:, tl, cc * 512:(cc + 1) * 512], pp[:])
                    else:
                        nc.scalar.copy(
                            obc[:, tl, cc * 512:(cc + 1) * 512], pp[:])
                # per-t DMA: overlaps the next t's copies (trims the
                # serial tail after the final chunk's proj)
                nc.sync.dma_start(outT_r[c][:, tl, :], obc[:, tl, :])

            def emit_proj(c):
                obc = ostage.tile([128, 4, C], F32)
                for tl in range(4):
                    emit_proj_t(c, tl, obc)

            xchs = {}

            def issue_xch_dma(c):
                cs = slice(c * 512, (c + 1) * 512)
                xch = xstream.tile([128, NKC, 512], F32R, name="xch")
                xchs[c] = xch
                if c == 0:
                    nc.sync.dma_start(wqkv[:, 0:4, :], wqkv_r[:, 0:4, :])
                    nc.sync.dma_start(xch[:, 0:4, :], xhatT_r[:, 0:4, cs])
                    nc.sync.dma_start(wqkv[:, 4:8, :], wqkv_r[:, 4:8, :])
                    nc.sync.dma_start(xch[:, 4:8, :], xhatT_r[:, 4:8, cs])
                    load_consts()
                else:
                    nc.sync.dma_start(xch[:], xhatT_r[:, :, cs])
                    if c == 1:
                        nc.sync.dma_start(wproj[:], d_wproj.ap())

            def emit_qkv_unit(c, g):
                # one of q/k/v for chunk c: 8 matmuls + swap/rope on DVE
                cs = slice(c * 512, (c + 1) * 512)
                dsts = (qhp, khp, None)[g]
                xch = xchs[c]
                ps = psA.tile([128, 512], F32, tag='a')
                for k in range(NKC):
                    nc.tensor.matmul(
                        ps[:], wqkv[:, k, g * D2:(g + 1) * D2],
                        xch[:, k, :],
                        start=(k == 0), stop=(k == NKC - 1))
                if dsts is None:
                    nc.scalar.copy(v2[:, cs], ps[:])
                    return
                # swap 32-partition halves within each 64-row head block
                # (Pool/GpSimd cannot read PSUM: vector/scalar only)
                dsw = work.tile([D2, 512], F32, tag="dsw")
                for h in range(HPC):
                    b = h * HD
                    if h == 0:
                        nc.vector.tensor_copy(dsw[b:b + 32, :], ps[b + 32:b + 64, :])
                        nc.vector.tensor_copy(dsw[b + 32:b + 64, :], ps[b:b + 32, :])
                    else:
                        nc.scalar.copy(dsw[b:b + 32, :], ps[b + 32:b + 64, :])
                        nc.scalar.copy(dsw[b + 32:b + 64, :], ps[b:b + 32, :])
                t1 = work.tile([D2, 512], F32, tag="rope1")
                t2 = work.tile([D2, 512], F32, tag="rope2")
                nc.vector.tensor_mul(t1[:], ps[:], ctab[:, cs])
                nc.vector.tensor_mul(t2[:], dsw[:], stab[:, cs])
                for h in range(HPC):
                    b = h * HD
                    nc.vector.tensor_add(dsts[h][0:HD, cs], t1[b:b + HD, :],
                                         t2[b:b + HD, :])

            def emit_vtrans_unit(c):
                for j in range(4 * c, 4 * (c + 1)):
                    pst_full = psA.tile([128, 512], F32, tag='a', name='pst')
                    pst = pst_full[:, :128]
                    nc.tensor.transpose(pst[:], v2[:, j * 128:(j + 1) * 128],
                                        identV[:])
                    nc.vector.tensor_copy(vprime[0][:, j, 0:HD], pst[:, 0:HD])
                    nc.vector.tensor_copy(vprime[1][:, j, 0:HD],
                                          pst[:, HD:2 * HD])

            # ---- startup: chunk 0 QKV/rope/V' with nothing to hide behind
            issue_xch_dma(0)
            for g in range(3):
                emit_qkv_unit(0, g)
            emit_vtrans_unit(0)

            for c in range(TT):
                cs = slice(c * 512, (c + 1) * 512)
                if c + 1 < TT:
                    issue_xch_dma(c + 1)

                # filler units: next chunk's QKV/V' + prior chunk's proj,
                # spread through this chunk's S/AV stream so the in-order
                # PE never starves while ACT (exp) catches up
                units = []
                if c + 1 < TT:
                    units += [lambda g=g: emit_qkv_unit(c + 1, g)
                              for g in range(3)]
                    units.append(lambda: emit_vtrans_unit(c + 1))

                njt = 4 * (c + 1)
                po = [psO.tile([HD + 1, 512], F32, tag=f'o{h}', name=f'po{h}')
                      for h in range(HPC)]
                ets = [[], []]
                etbase = [[], []]

                def emit_av(h, j):
                    m = j - 4 * c
                    if m >= 2:
                        # masked cols were skipped by the sliced exp; skip
                        # them here too (their contribution is exactly 0)
                        mb = 128 * m
                        ett, off = etbase[h][j]
                        nc.tensor.matmul(
                            po[h][:, mb:512], vprime[h][:, j, :],
                            ett[:, off + mb:off + 512],
                            start=(j == 0), stop=(j == njt - 1),
                            skip_group_check=True)
                    else:
                        nc.tensor.matmul(
                            po[h][:], vprime[h][:, j, :], ets[h][j],
                            start=(j == 0), stop=(j == njt - 1),
                            skip_group_check=True)

                nu = len(units)
                fire_list = [(max(1, ((u + 1) * njt) // (nu + 1)), u)
                             for u in range(nu)]
                fired = set()
                for j in range(njt):
                    # both heads' S tiles share one 2-bank PSUM tile so a
                    # single 1024-wide exp amortizes the ACT fixed costs
                    pspair = psS.tile([128, 1024], F32, tag='s')
                    m = j - 4 * c
                    for h in range(HPC):
                        hb5 = h * 512
                        nc.tensor.matmul(
                            pspair[:, hb5:hb5 + 512],
                            khp[h][:, j * 128:(j + 1) * 128],
                            qhp[h][:, cs], start=True, stop=(m < 0),
                            skip_group_check=(m >= 0))
                        if m >= 0:  # diagonal tile: additive causal mask
                            # only cols < 128(m+1) contain -1e30; shorten the
                            # accumulate (>=256 keeps f32r at 1 cyc/row)
                            W = max(256, 128 * (m + 1))
                            nc.tensor.matmul(pspair[:, hb5:hb5 + W], identR[:],
                                             maskadd[:, m, 0:W],
                                             start=False, stop=True,
                                             skip_group_check=True)
                    et = estrip.tile([128, 1024], BF16)
                    if m >= 2:
                        # cols < 128m are fully masked (exp -> 0) and the AV
                        # skips them below; exp only the live columns
                        mb = 128 * m
                        for h in range(HPC):
                            nc.scalar.activation(
                                et[:, h * 512 + mb:(h + 1) * 512],
                                pspair[:, h * 512 + mb:(h + 1) * 512],
                                mybir.ActivationFunctionType.Exp,
                                scale=float(1.0 / np.sqrt(HD)))
                    else:
                        nc.scalar.activation(et[:], pspair[:],
                                             mybir.ActivationFunctionType.Exp,
                                             scale=float(1.0 / np.sqrt(HD)))
                    for h in range(HPC):
                        ets[h].append(et[:, h * 512:(h + 1) * 512])
                        etbase[h].append((et, h * 512))
                    if j >= LAG:
                        for h in range(HPC):
                            emit_av(h, j - LAG)
                    for jj, u in fire_list:
                        if jj <= j and u not in fired:
                            fired.add(u)
                            units[u]()
                for u in range(nu):
                    if u not in fired:
                        units[u]()
                # AV drain interleaved with the prior chunk's proj t-tiles:
                # each proj piece covers the wait for the next trailing exp
                def emit_normalize(h):
                    # normalize: yhat = po[:64] * (1/po[64]) broadcast
                    # (copy denom to partition 0 first: the custom-DVE
                    # reciprocal does not honor input partition offsets)
                    dcp = small.tile([1, 512], F32, tag="dcp")
                    nc.vector.tensor_copy(dcp[:], po[h][HD:HD + 1, :])
                    rec = small.tile([1, 512], F32, tag="rec")
                    nc.vector.reciprocal_approx_fast(rec[:], dcp[:])
                    rb = small.tile([HD, 512], F32, tag="recb")
                    nc.gpsimd.partition_broadcast(rb[:], rec[:])
                    nc.vector.tensor_mul(yhat[h * HD:(h + 1) * HD, cs],
                                         po[h][0:HD, :], rb[:])

                projt = list(range(4)) if c >= 1 else []
                obc = (ostage.tile([128, 4, C], F32, name="obc")
                       if projt else None)
                for j in range(max(0, njt - LAG), njt):
                    if projt:
                        emit_proj_t(c - 1, projt.pop(0), obc)
                    for h in range(HPC):
                        emit_av(h, j)
                        if j == njt - 1:
                            emit_normalize(h)
                for tl in projt:
                    emit_proj_t(c - 1, tl, obc)
            emit_proj(TT - 1)

    nc.compile()
    _nc_cache["attn"] = nc
    return nc


# --------------------------------------------------------------------------
# Launch B: MoE expert (1 expert per core, host-dispatched tokens)
# --------------------------------------------------------------------------

def _chunks(cap):
    # split into equal-ish chunks <= 512 (PSUM bank limit). Equal widths
    # beat (512, small): per-matmul ldweights overhead dominates small
    # moving dims.
    n = (cap + 511) // 512
    base = cap // n
    ch = []
    off = 0
    for i in range(n):
        w = base + (1 if i < cap - base * n else 0)
        ch.append((off, w))
        off += w
    return ch


def build_moe(cap):
    key = ("moe", cap)
    if key in _nc_cache:
        return _nc_cache[key]
    nc = bacc.Bacc("TRN2", target_bir_lowering=False, debug=False,
                   num_devices=NCORES)

    NKC = C // 128    # 8
    NI = HFF // 128   # 32
    NJ = C // 128     # 8
    CH = _chunks(cap)
    DR = mybir.MatmulPerfMode.DoubleRow

    d_xgT = nc.dram_tensor("xgT", [C, cap], F8, kind="ExternalInput")
    # host-pretiled layouts: [block, 128p, k, 128] with contiguous rows;
    # weights pre-scaled by S_W (wg, wd) / S_U (wu) and cast to fp8e4.
    d_wg4 = nc.dram_tensor("wg4", [NI, 128, NKC, 128], F8, kind="ExternalInput")
    d_wu4 = nc.dram_tensor("wu4", [NI, 128, NKC, 128], F8, kind="ExternalInput")
    d_wd4 = nc.dram_tensor("wd4", [NJ, 128, NI, 128], F8, kind="ExternalInput")
    d_yT = nc.dram_tensor("yT", [C, cap], F32, kind="ExternalOutput")

    with tile.TileContext(nc) as tc:
        with tc.tile_pool(name="xg", bufs=1) as xgp, \
             tc.tile_pool(name="hsb", bufs=1) as hsbp, \
             tc.tile_pool(name="hst", bufs=3) as hstp, \
             tc.tile_pool(name="wload", bufs=3) as wload, \
             tc.tile_pool(name="wdload", bufs=3) as wdload, \
             tc.tile_pool(name="ob", bufs=3) as obp, \
             tc.tile_pool(name="psG", bufs=3, space="PSUM") as psG, \
             tc.tile_pool(name="psY", bufs=2, space="PSUM") as psY:

            # xg as one [128, k, n] tile so DoubleRow can take k-pair slices;
            # DMA split per k-pair, with i=0's weights interleaved right
            # after the first pair so the first matmul starts early
            xg = xgp.tile([128, NKC, cap], F8)
            xgT_r = d_xgT.ap().rearrange("(ko p) n -> p ko n", p=128)
            nc.sync.dma_start(xg[:, 0:2, :], xgT_r[:, 0:2, :])
            w0g = wload.tile([128, NKC, 128], F8, tag="wg", name="wg0")
            nc.sync.dma_start(w0g[:], d_wg4.ap()[0])
            w0u = wload.tile([128, NKC, 128], F8, tag="wu", name="wu0")
            nc.sync.dma_start(w0u[:], d_wu4.ap()[0])
            for t in range(1, NKC // 2):
                nc.sync.dma_start(xg[:, 2 * t:2 * t + 2, :],
                                  xgT_r[:, 2 * t:2 * t + 2, :])

            hsb = hsbp.tile([128, NI, cap], F8)

            # Phase 1: h = silu(wg.T@xg) * (wu.T@xg); fp8 DoubleRow over
            # k-tile pairs (contraction 256/instr).
            wdts = {}
            for i in range(NI):
                wgt = wload.tile([128, NKC, 128], F8, tag="wg")
                nc.sync.dma_start(wgt[:], d_wg4.ap()[i])
                wut = wload.tile([128, NKC, 128], F8, tag="wu")
                nc.sync.dma_start(wut[:], d_wu4.ap()[i])
                if i >= NI - 3:  # prefetch phase-2 weights under phase-1 tail
                    j = i - (NI - 3)
                    wdts[j] = wdload.tile([128, NI, 128], F8, tag="wd",
                                          name=f"wdpre{j}")
                    nc.sync.dma_start(wdts[j][:], d_wd4.ap()[j])
                for (off, n) in CH:
                    pg = psG.tile([128, 512], F32, tag="pg")
                    pu = psG.tile([128, 512], F32, tag="pu")
                    for t in range(NKC // 2):
                        nc.tensor.matmul(pg[:, :n], wgt[:, 2 * t:2 * t + 2, :],
                                         xg[:, 2 * t:2 * t + 2, off:off + n],
                                         start=(t == 0), stop=(t == NKC // 2 - 1),
                                         perf_mode=DR)
                    for t in range(NKC // 2):
                        nc.tensor.matmul(pu[:, :n], wut[:, 2 * t:2 * t + 2, :],
                                         xg[:, 2 * t:2 * t + 2, off:off + n],
                                         start=(t == 0), stop=(t == NKC // 2 - 1),
                                         perf_mode=DR)
                    hs = hstp.tile([128, 512], F32)
                    nc.scalar.activation(hs[:, :n], pg[:, :n],
                                         mybir.ActivationFunctionType.Silu,
                                         scale=float(1.0 / S_W))
                    # hsb = silu(g) * (S_U*u) = S_H*h  (S_H == S_U)
                    nc.vector.tensor_mul(hsb[:, i, off:off + n],
                                         hs[:, :n], pu[:, :n])

            # Phase 2: yT[j] = sum_i wd4[j][:, i].T @ h[i]; DR over i-pairs
            for j in range(NJ):
                if j in wdts:
                    wdt = wdts.pop(j)
                else:
                    wdt = wdload.tile([128, NI, 128], F8, tag="wd")
                    nc.sync.dma_start(wdt[:], d_wd4.ap()[j])
                for (off, n) in CH:
                    py = psY.tile([128, 512], F32)
                    for t in range(NI // 2):
                        nc.tensor.matmul(py[:, :n], wdt[:, 2 * t:2 * t + 2, :],
                                         hsb[:, 2 * t:2 * t + 2, off:off + n],
                                         start=(t == 0), stop=(t == NI // 2 - 1),
                                         perf_mode=DR)
                    ob = obp.tile([128, 512], F32)
                    if j % 2 == 0:
                        nc.vector.tensor_copy(ob[:, :n], py[:, :n])
                    else:
                        nc.scalar.copy(ob[:, :n], py[:, :n])
                    nc.sync.dma_start(
                        d_yT.ap()[j * 128:(j + 1) * 128, off:off + n],
                        ob[:, :n])

    nc.compile()
    _nc_cache[key] = nc
    return nc


# --------------------------------------------------------------------------
# Host orchestration
# --------------------------------------------------------------------------

def _rope_tables():
    inv_freq = 1.0 / (10000.0 ** (np.arange(0, HD, 2, dtype=np.float32) / HD))
    t = np.arange(T, dtype=np.float32)
    freqs = np.einsum("i,j->ij", t, inv_freq).astype(np.float32)   # [T, 32]
    emb = np.concatenate([freqs, freqs], axis=-1)                   # [T, 64]
    cos = np.cos(emb).astype(np.float32)
    sin = np.sin(emb).astype(np.float32)
    cosT = np.ascontiguousarray(cos.T)                              # [64, T]
    # stabA pairs with the partition-swapped operand: d<32 -> -sin, d>=32 -> +sin
    sinA = np.empty((HD, T), np.float32)
    sinA[:32] = -sin.T[:32]
    sinA[32:] = sin.T[32:]
    ctab = np.concatenate([cosT] * HPC, axis=0)                     # [128, T]
    stab = np.concatenate([sinA] * HPC, axis=0)
    return ctab, stab


def _causal_masks():
    # additive mask[m, p, f] = 0 where query f sees key (p + 128*m) within
    # the diagonal 512-block, else -1e30 (absorbs s in fp32, exp -> 0).
    f = np.arange(512)[None, :]
    p = np.arange(128)[:, None]
    m4 = np.stack([np.where(f >= p + 128 * m, 0.0, -1e30)
                   for m in range(4)]).astype(np.float32)            # [4,128,512]
    return m4


def _run(nc, in_maps, trace=False, tmpdir=None):
    return run_bass_kernel_spmd(nc, in_maps, list(range(NCORES)),
                                trace=trace, tmpdir=tmpdir)


def kernel(x, norm1_w, norm2_w, qkv_w, proj_w, router_w, wg, wu, wd,
           _trace=False, _stats=None):
    x = np.asarray(x, np.float32)
    B = x.shape[0]
    xf = x.reshape(T, C)

    # ---- host: rms_norm 1 (norm1_w folded into qkv weights) ----
    ms = np.mean(xf * xf, axis=-1, keepdims=True)
    xhat = xf / np.sqrt(ms + EPS)
    xhatT = np.ascontiguousarray(xhat.T)                    # [C, T]

    ctab, stab = _rope_tables()
    masks = _causal_masks()

    qkv_s = (np.asarray(qkv_w, np.float32) * np.asarray(norm1_w, np.float32)[None, :])
    proj = np.asarray(proj_w, np.float32)

    nc_a = build_attention()
    ident = np.eye(128, dtype=np.float32)
    in_maps = []
    for core in range(NCORES):
        h0 = core * HPC
        rows = []
        for g in range(3):  # q, k, v
            rows.append(qkv_s[g * C + h0 * HD: g * C + (h0 + HPC) * HD, :])
        wqkv_c = np.ascontiguousarray(np.concatenate(rows, axis=0).T)  # [C, 384]
        wproj_c = np.ascontiguousarray(proj[:, h0 * HD:(h0 + HPC) * HD].T)  # [128, C]
        in_maps.append({
            "xhatT": xhatT, "wqkv": wqkv_c, "wproj": wproj_c,
            "ctab": ctab, "stab": stab, "mask": masks,
            "identV": ident, "identR": ident,
        })
    res_a = _run(nc_a, in_maps, trace=_trace,
                 tmpdir="/tmp/trace_attn" if _trace else None)
    attn = np.zeros((T, C), np.float32)
    for core in range(NCORES):
        attn += res_a.results[core]["attn_part"]

    xa = xf + attn

    # ---- host: rms_norm 2 + router + top-2 dispatch ----
    ms2 = np.mean(xa * xa, axis=-1, keepdims=True)
    x2 = (xa / np.sqrt(ms2 + EPS)) * np.asarray(norm2_w, np.float32)[None, :]
    logits = x2 @ np.asarray(router_w, np.float32).T        # [T, E]
    topi = np.argsort(-logits, axis=-1)[:, :2]              # [T, 2]
    topv = np.take_along_axis(logits, topi, axis=-1)
    mx = topv.max(axis=-1, keepdims=True)
    ex = np.exp(topv - mx)
    wts = ex / ex.sum(axis=-1, keepdims=True)               # [T, 2]

    idxs, gts = [], []
    for e in range(E):
        sel = np.nonzero((topi == e).any(axis=-1))[0]
        gsel = np.where(topi[sel, 0] == e, wts[sel, 0], wts[sel, 1])
        idxs.append(sel)
        gts.append(gsel.astype(np.float32))
    maxload = max(len(s) for s in idxs)
    cap = max(256, ((maxload + 127) // 128) * 128)

    import ml_dtypes
    F8NP = ml_dtypes.float8_e4m3

    nc_b = build_moe(cap)
    NI, NJ, NKC = HFF // 128, C // 128, C // 128
    in_maps_b = []
    for e in range(E):
        xgT = np.zeros((C, cap), F8NP)
        xgT[:, :len(idxs[e])] = x2[idxs[e]].T.astype(F8NP)
        wg_e = np.asarray(wg[e], np.float32) * S_W
        wu_e = np.asarray(wu[e], np.float32) * S_U
        wd_e = np.asarray(wd[e], np.float32) * S_W
        in_maps_b.append({
            "xgT": xgT,
            "wg4": np.ascontiguousarray(
                wg_e.reshape(NI, 128, NKC, 128).transpose(0, 3, 2, 1)
            ).astype(F8NP),
            "wu4": np.ascontiguousarray(
                wu_e.reshape(NI, 128, NKC, 128).transpose(0, 3, 2, 1)
            ).astype(F8NP),
            "wd4": np.ascontiguousarray(
                wd_e.reshape(NJ, 128, NI, 128).transpose(0, 3, 2, 1)
            ).astype(F8NP),
        })
    res_b = _run(nc_b, in_maps_b, trace=_trace,
                 tmpdir="/tmp/trace_moe" if _trace else None)

    out = xa.copy()
    for e in range(E):
        yT = res_b.results[e]["yT"]                          # [C, cap]
        n = len(idxs[e])
        out[idxs[e]] += yT[:, :n].T * (gts[e] / (S_W * S_H))[:, None]

    if _stats is not None:
        _stats["attn_ns"] = res_a.exec_time_ns
        _stats["moe_ns"] = res_b.exec_time_ns
        _stats["cap"] = cap
    return out.reshape(B, T, C)



# revision 70
# speedup vs baseline: 1.0068x; 1.0068x over previous
"""Trainium2 Bass kernel for nn_Block_30262339567868 (attention + top-2 MoE block).

Self-contained: takes FULL inputs, shards across 8 NeuronCores internally,
returns the FULL output.

Sharding:
  - Attention: head-parallel (16 heads / 8 cores = 2 heads per core), each core
    produces a partial projection output; host sums partials.
  - MoE: expert-parallel (8 experts / 8 cores), host-side token dispatch
    (gather to per-expert capacity buffers) and gate-weighted scatter-add.
Matmuls run as float32r (tf32-class) except the attention inner (exp(S), V in
bf16). All matmuls use a uniform K=128 contraction (S is zero-padded) — the PE
pays ~200ns per contraction-size switch.
"""

import numpy as np

import concourse.bass as bass
import concourse.mybir as mybir
import concourse.tile as tile
from concourse import bacc
from concourse.bass_utils import run_bass_kernel_spmd
from concourse.masks import make_identity

# Problem shapes (hardcoded per contract)
T = 2048
C = 1024
E = 8
HFF = 4096
NH = 16
HD = 64
NCORES = 8
HPC = NH // NCORES  # heads per core = 2
EPS = 1e-6

F32 = mybir.dt.float32
F32R = mybir.dt.float32r
BF16 = mybir.dt.bfloat16
F8 = mybir.dt.float8e4

# MoE fp8 scale scheme: weights pre-scaled before e4m3 cast; the silu
# input is descaled on-chip (activation scale), the rest is folded into
# the host-side gate weights.
S_W = 32.0   # wg, wd scale
S_U = 16.0   # wu scale == S_H (so h_fp8 = silu(g) * pu directly)
S_H = 16.0

_nc_cache = {}


# --------------------------------------------------------------------------
# Launch A: attention (head-sharded)
# --------------------------------------------------------------------------

def build_attention():
    if "attn" in _nc_cache:
        return _nc_cache["attn"]
    nc = bacc.Bacc("TRN2", target_bir_lowering=False, debug=False,
                   num_devices=NCORES)

    d_xhatT = nc.dram_tensor("xhatT", [C, T], F32R, kind="ExternalInput")
    d_wqkv = nc.dram_tensor("wqkv", [C, 3 * HPC * HD], F32R, kind="ExternalInput")
    d_wproj = nc.dram_tensor("wproj", [HPC * HD, C], F32R, kind="ExternalInput")
    d_ctab = nc.dram_tensor("ctab", [HPC * HD, T], F32, kind="ExternalInput")
    d_stab = nc.dram_tensor("stab", [HPC * HD, T], F32, kind="ExternalInput")
    # additive causal masks for the 4 diagonal k-tiles of a 512-query chunk
    d_mask = nc.dram_tensor("mask", [4, 128, 512], F32R, kind="ExternalInput")
    d_identV = nc.dram_tensor("identV", [128, 128], F32, kind="ExternalInput")
    d_identR = nc.dram_tensor("identR", [128, 128], F32R, kind="ExternalInput")
    d_out = nc.dram_tensor("attn_part", [T, C], F32, kind="ExternalOutput")

    TT = T // 512        # 4 tq chunks
    NTK = T // 128       # 16 tk tiles
    D2 = HPC * HD        # 128
    NKC = C // 128       # 8

    with tile.TileContext(nc) as tc:
        with tc.tile_pool(name="big", bufs=1) as big, \
             tc.tile_pool(name="consts", bufs=1) as consts, \
             tc.tile_pool(name="xstream", bufs=2) as xstream, \
             tc.tile_pool(name="work", bufs=1) as work, \
             tc.tile_pool(name="small", bufs=2) as small, \
             tc.tile_pool(name="ostage", bufs=2) as ostage, \
             tc.tile_pool(name="estrip", bufs=6) as estrip, \
             tc.tile_pool(name="psA", bufs=2, space="PSUM") as psA, \
             tc.tile_pool(name="psS", bufs=2, space="PSUM") as psS, \
             tc.tile_pool(name="psO", bufs=1, space="PSUM") as psO:

            # ---- constants ----
            xhatT_r = d_xhatT.ap().rearrange("(ko p) t -> p ko t", p=128)
            wqkv_r = d_wqkv.ap().rearrange("(ko p) m -> p ko m", p=128)
            wqkv = consts.tile([128, NKC, 3 * D2], F32R)
            wproj = consts.tile([D2, C], F32R)
            ctab = consts.tile([D2, T], F32)
            stab = consts.tile([D2, T], F32)
            maskadd = consts.tile([128, 4, 512], F32R)
            identV = consts.tile([128, 128], F32)
            identR = consts.tile([128, 128], F32R)

            def load_consts():  # queued after chunk 0's x data
                nc.sync.dma_start(ctab[:], d_ctab.ap())
                nc.sync.dma_start(stab[:], d_stab.ap())
                nc.sync.dma_start(identV[:], d_identV.ap())
                nc.sync.dma_start(identR[:], d_identR.ap())
                nc.sync.dma_start(maskadd[:],
                                  d_mask.ap().rearrange("m p f -> p m f"))

            # per-head q/k, real data in rows [0:64], zero-padded [64:128]
            # (uniform K=128 contraction: PE pays ~200ns per size switch)
            qhp = [big.tile([128, T], F32R, name=f"qhp{h}") for h in range(HPC)]
            khp = [big.tile([128, T], F32R, name=f"khp{h}") for h in range(HPC)]
            v2 = big.tile([D2, T], F32)
            vprime = [big.tile([128, NTK, HD + 1], BF16, name=f"vp{h}")
                      for h in range(HPC)]
            yhat = big.tile([D2, T], F32R)
            for h in range(HPC):
                nc.any.memset(vprime[h][:, :, HD:HD + 1], 1.0)
            # zero the pad rows (hidden under the first x-chunk DMA);
            # memset on f32r is invalid ISA -> bounce zeros via an F32 tile
            zsrc = work.tile([HD, T], F32, tag="zsrc")
            nc.gpsimd.memset(zsrc[:], 0.0)
            nc.vector.tensor_copy(qhp[0][HD:, :], zsrc[:])
            nc.vector.tensor_copy(khp[0][HD:, :], zsrc[:])
            nc.scalar.copy(qhp[1][HD:, :], zsrc[:])
            nc.scalar.copy(khp[1][HD:, :], zsrc[:])

            LAG = 3
            pending_proj = []
            # output staged per chunk: [128, 4 t-tiles, C] -> one 2MB DMA
            outT_r = d_out.ap().rearrange("(b tt p) c -> b p tt c",
                                          tt=4, p=128)

            def emit_proj_t(c, tl, obc, dve_only=False):
                # dve_only: inside the AV drain the Scalar engine must finish
                # the trailing exps -- keep proj's bounce copies off it there
                t = 4 * c + tl
                for cc in range(C // 512):
                    pp = psA.tile([128, 512], F32, tag='a')
                    nc.tensor.matmul(pp[:], yhat[:, t * 128:(t + 1) * 128],
                                     wproj[:, cc * 512:(cc + 1) * 512],
                                     start=True, stop=True)
                    if dve_only or (t + cc) % 2 == 0:
                        nc.vector.tensor_copy(
                            obc[:, tl, cc * 512:(cc + 1) * 512], pp[:])
                    else:
                        nc.scalar.copy(
                            obc[:, tl, cc * 512:(cc + 1) * 512], pp[:])
                # per-t DMA: overlaps the next t's copies (trims the
                # serial tail after the final chunk's proj)
                nc.sync.dma_start(outT_r[c][:, tl, :], obc[:, tl, :])

            def emit_proj(c):
                obc = ostage.tile([128, 4, C], F32)
                for tl in range(4):
                    emit_proj_t(c, tl, obc)

            xchs = {}

            def issue_xch_dma(c):
                cs = slice(c * 512, (c + 1) * 512)
                xch = xstream.tile([128, NKC, 512], F32R, name="xch")
                xchs[c] = xch
                if c == 0:
                    nc.sync.dma_start(wqkv[:, 0:4, :], wqkv_r[:, 0:4, :])
                    nc.sync.dma_start(xch[:, 0:4, :], xhatT_r[:, 0:4, cs])
                    nc.sync.dma_start(wqkv[:, 4:8, :], wqkv_r[:, 4:8, :])
                    nc.sync.dma_start(xch[:, 4:8, :], xhatT_r[:, 4:8, cs])
                    load_consts()
                else:
                    nc.sync.dma_start(xch[:], xhatT_r[:, :, cs])
                    if c == 1:
                        nc.sync.dma_start(wproj[:], d_wproj.ap())

            def emit_qkv_unit(c, g):
                # one of q/k/v for chunk c: 8 matmuls + swap/rope on DVE
                cs = slice(c * 512, (c + 1) * 512)
                dsts = (qhp, khp, None)[g]
                xch = xchs[c]
                ps = psA.tile([128, 512], F32, tag='a')
                for k in range(NKC):
                    nc.tensor.matmul(
                        ps[:], wqkv[:, k, g * D2:(g + 1) * D2],
                        xch[:, k, :],
                        start=(k == 0), stop=(k == NKC - 1))
                if dsts is None:
                    nc.scalar.copy(v2[:, cs], ps[:])
                    return
                # swap 32-partition halves within each 64-row head block
                # (Pool/GpSimd cannot read PSUM: vector/scalar only)
                dsw = work.tile([D2, 512], F32, tag="dsw")
                for h in range(HPC):
                    b = h * HD
                    if h == 0:
                        nc.vector.tensor_copy(dsw[b:b + 32, :], ps[b + 32:b + 64, :])
                        nc.vector.tensor_copy(dsw[b + 32:b + 64, :], ps[b:b + 32, :])
                    else:
                        nc.scalar.copy(dsw[b:b + 32, :], ps[b + 32:b + 64, :])
                        nc.scalar.copy(dsw[b + 32:b + 64, :], ps[b:b + 32, :])
                t1 = work.tile([D2, 512], F32, tag="rope1")
                t2 = work.tile([D2, 512], F32, tag="rope2")
                nc.vector.tensor_mul(t1[:], ps[:], ctab[:, cs])
                nc.vector.tensor_mul(t2[:], dsw[:], stab[:, cs])
                for h in range(HPC):
                    b = h * HD
                    nc.vector.tensor_add(dsts[h][0:HD, cs], t1[b:b + HD, :],
                                         t2[b:b + HD, :])

            def emit_vtrans_unit(c):
                for j in range(4 * c, 4 * (c + 1)):
                    pst_full = psA.tile([128, 512], F32, tag='a', name='pst')
                    pst = pst_full[:, :128]
                    nc.tensor.transpose(pst[:], v2[:, j * 128:(j + 1) * 128],
                                        identV[:])
                    nc.vector.tensor_copy(vprime[0][:, j, 0:HD], pst[:, 0:HD])
                    nc.vector.tensor_copy(vprime[1][:, j, 0:HD],
                                          pst[:, HD:2 * HD])

            # ---- startup: chunk 0 QKV/rope/V' with nothing to hide behind
            issue_xch_dma(0)
            for g in range(3):
                emit_qkv_unit(0, g)
            emit_vtrans_unit(0)

            for c in range(TT):
                cs = slice(c * 512, (c + 1) * 512)
                if c + 1 < TT:
                    issue_xch_dma(c + 1)

                # filler units: next chunk's QKV/V' + prior chunk's proj,
                # spread through this chunk's S/AV stream so the in-order
                # PE never starves while ACT (exp) catches up
                units = []
                if c + 1 < TT:
                    units += [lambda g=g: emit_qkv_unit(c + 1, g)
                              for g in range(3)]
                    units.append(lambda: emit_vtrans_unit(c + 1))

                njt = 4 * (c + 1)
                po = [psO.tile([HD + 1, 512], F32, tag=f'o{h}', name=f'po{h}')
                      for h in range(HPC)]
                ets = [[], []]
                etbase = [[], []]

                def emit_av(h, j):
                    m = j - 4 * c
                    if m >= 2:
                        # masked cols were skipped by the sliced exp; skip
                        # them here too (their contribution is exactly 0)
                        mb = 128 * m
                        ett, off = etbase[h][j]
                        nc.tensor.matmul(
                            po[h][:, mb:512], vprime[h][:, j, :],
                            ett[:, off + mb:off + 512],
                            start=(j == 0), stop=(j == njt - 1),
                            skip_group_check=True)
                    else:
                        nc.tensor.matmul(
                            po[h][:], vprime[h][:, j, :], ets[h][j],
                            start=(j == 0), stop=(j == njt - 1),
                            skip_group_check=True)

                nu = len(units)
                fire_list = [(max(1, ((u + 1) * njt) // (nu + 1)), u)
                             for u in range(nu)]
                fired = set()
                for j in range(njt):
                    # both heads' S tiles share one 2-bank PSUM tile so a
                    # single 1024-wide exp amortizes the ACT fixed costs
                    pspair = psS.tile([128, 1024], F32, tag='s')
                    m = j - 4 * c
                    for h in range(HPC):
                        hb5 = h * 512
                        nc.tensor.matmul(
                            pspair[:, hb5:hb5 + 512],
                            khp[h][:, j * 128:(j + 1) * 128],
                            qhp[h][:, cs], start=True, stop=(m < 0),
                            skip_group_check=(m >= 0))
                        if m >= 0:  # diagonal tile: additive causal mask
                            # only cols < 128(m+1) contain -1e30; shorten the
                            # accumulate (>=256 keeps f32r at 1 cyc/row)
                            W = max(256, 128 * (m + 1))
                            nc.tensor.matmul(pspair[:, hb5:hb5 + W], identR[:],
                                             maskadd[:, m, 0:W],
                                             start=False, stop=True,
                                             skip_group_check=True)
                    et = estrip.tile([128, 1024], BF16)
                    if m >= 2:
                        # cols < 128m are fully masked (exp -> 0) and the AV
                        # skips them below; exp only the live columns
                        mb = 128 * m
                        for h in range(HPC):
                            nc.scalar.activation(
                                et[:, h * 512 + mb:(h + 1) * 512],
                                pspair[:, h * 512 + mb:(h + 1) * 512],
                                mybir.ActivationFunctionType.Exp,
                                scale=float(1.0 / np.sqrt(HD)))
                    else:
                        nc.scalar.activation(et[:], pspair[:],
                                             mybir.ActivationFunctionType.Exp,
                                             scale=float(1.0 / np.sqrt(HD)))
                    for h in range(HPC):
                        ets[h].append(et[:, h * 512:(h + 1) * 512])
                        etbase[h].append((et, h * 512))
                    if j >= LAG:
                        for h in range(HPC):
                            emit_av(h, j - LAG)
                    for jj, u in fire_list:
                        if jj <= j and u not in fired:
                            fired.add(u)
                            units[u]()
                for u in range(nu):
                    if u not in fired:
                        units[u]()
                # AV drain interleaved with the prior chunk's proj t-tiles:
                # each proj piece covers the wait for the next trailing exp
                def emit_normalize(h):
                    # normalize: yhat = po[:64] * (1/po[64]) broadcast
                    # (copy denom to partition 0 first: the custom-DVE
                    # reciprocal does not honor input partition offsets)
                    dcp = small.tile([1, 512], F32, tag="dcp")
                    nc.vector.tensor_copy(dcp[:], po[h][HD:HD + 1, :])
                    rec = small.tile([1, 512], F32, tag="rec")
                    nc.vector.reciprocal_approx_fast(rec[:], dcp[:])
                    rb = small.tile([HD, 512], F32, tag="recb")
                    nc.gpsimd.partition_broadcast(rb[:], rec[:])
                    nc.vector.tensor_mul(yhat[h * HD:(h + 1) * HD, cs],
                                         po[h][0:HD, :], rb[:])

                projt = list(range(4)) if c >= 1 else []
                obc = (ostage.tile([128, 4, C], F32, name="obc")
                       if projt else None)
                for j in range(max(0, njt - LAG), njt):
                    if projt:
                        emit_proj_t(c - 1, projt.pop(0), obc)
                    for h in range(HPC):
                        emit_av(h, j)
                        if j == njt - 1:
                            emit_normalize(h)
                for tl in projt:
                    emit_proj_t(c - 1, tl, obc)
            emit_proj(TT - 1)

    nc.compile()
    _nc_cache["attn"] = nc
    return nc


# --------------------------------------------------------------------------
# Launch B: MoE expert (1 expert per core, host-dispatched tokens)
# --------------------------------------------------------------------------

def _chunks(cap):
    # split into equal-ish chunks <= 512 (PSUM bank limit). Equal widths
    # beat (512, small): per-matmul ldweights overhead dominates small
    # moving dims.
    n = (cap + 511) // 512
    base = cap // n
    ch = []
    off = 0
    for i in range(n):
        w = base + (1 if i < cap - base * n else 0)
        ch.append((off, w))
        off += w
    return ch


def build_moe(cap):
    key = ("moe", cap)
    if key in _nc_cache:
        return _nc_cache[key]
    nc = bacc.Bacc("TRN2", target_bir_lowering=False, debug=False,
                   num_devices=NCORES)

    NKC = C // 128    # 8
    NI = HFF // 128   # 32
    NJ = C // 128     # 8
    CH = _chunks(cap)
    DR = mybir.MatmulPerfMode.DoubleRow

    d_xgT = nc.dram_tensor("xgT", [C, cap], F8, kind="ExternalInput")
    # host-pretiled layouts: [block, 128p, k, 128] with contiguous rows;
    # weights pre-scaled by S_W (wg, wd) / S_U (wu) and cast to fp8e4.
    d_wg4 = nc.dram_tensor("wg4", [NI, 128, NKC, 128], F8, kind="ExternalInput")
    d_wu4 = nc.dram_tensor("wu4", [NI, 128, NKC, 128], F8, kind="ExternalInput")
    d_wd4 = nc.dram_tensor("wd4", [NJ, 128, NI, 128], F8, kind="ExternalInput")
    d_yT = nc.dram_tensor("yT", [C, cap], F32, kind="ExternalOutput")

    with tile.TileContext(nc) as tc:
        with tc.tile_pool(name="xg", bufs=1) as xgp, \
             tc.tile_pool(name="hsb", bufs=1) as hsbp, \
             tc.tile_pool(name="hst", bufs=3) as hstp, \
             tc.tile_pool(name="wload", bufs=3) as wload, \
             tc.tile_pool(name="wdload", bufs=3) as wdload, \
             tc.tile_pool(name="ob", bufs=3) as obp, \
             tc.tile_pool(name="psG", bufs=3, space="PSUM") as psG, \
             tc.tile_pool(name="psY", bufs=2, space="PSUM") as psY:

            # xg as one [128, k, n] tile so DoubleRow can take k-pair slices;
            # DMA split per k-pair, with i=0's weights interleaved right
            # after the first pair so the first matmul starts early
            xg = xgp.tile([128, NKC, cap], F8)
            xgT_r = d_xgT.ap().rearrange("(ko p) n -> p ko n", p=128)
            nc.sync.dma_start(xg[:, 0:2, :], xgT_r[:, 0:2, :])
            w0g = wload.tile([128, NKC, 128], F8, tag="wg", name="wg0")
            nc.sync.dma_start(w0g[:], d_wg4.ap()[0])
            w0u = wload.tile([128, NKC, 128], F8, tag="wu", name="wu0")
            nc.sync.dma_start(w0u[:], d_wu4.ap()[0])
            for t in range(1, NKC // 2):
                nc.sync.dma_start(xg[:, 2 * t:2 * t + 2, :],
                                  xgT_r[:, 2 * t:2 * t + 2, :])

            hsb = hsbp.tile([128, NI, cap], F8)

            # Phase 1: h = silu(wg.T@xg) * (wu.T@xg); fp8 DoubleRow over
            # k-tile pairs (contraction 256/instr).
            wdts = {}
            for i in range(NI):
                if i == 0:
                    wgt, wut = w0g, w0u  # already DMA'd with the first xg pair
                else:
                    wgt = wload.tile([128, NKC, 128], F8, tag="wg")
                    nc.sync.dma_start(wgt[:], d_wg4.ap()[i])
                    wut = wload.tile([128, NKC, 128], F8, tag="wu")
                    nc.sync.dma_start(wut[:], d_wu4.ap()[i])
                if i >= NI - 3:  # prefetch phase-2 weights under phase-1 tail
                    j = i - (NI - 3)
                    wdts[j] = wdload.tile([128, NI, 128], F8, tag="wd",
                                          name=f"wdpre{j}")
                    nc.sync.dma_start(wdts[j][:], d_wd4.ap()[j])
                for (off, n) in CH:
                    pg = psG.tile([128, 512], F32, tag="pg")
                    pu = psG.tile([128, 512], F32, tag="pu")
                    for t in range(NKC // 2):
                        nc.tensor.matmul(pg[:, :n], wgt[:, 2 * t:2 * t + 2, :],
                                         xg[:, 2 * t:2 * t + 2, off:off + n],
                                         start=(t == 0), stop=(t == NKC // 2 - 1),
                                         perf_mode=DR)
                    for t in range(NKC // 2):
                        nc.tensor.matmul(pu[:, :n], wut[:, 2 * t:2 * t + 2, :],
                                         xg[:, 2 * t:2 * t + 2, off:off + n],
                                         start=(t == 0), stop=(t == NKC // 2 - 1),
                                         perf_mode=DR)
                    hs = hstp.tile([128, 512], F32)
                    nc.scalar.activation(hs[:, :n], pg[:, :n],
                                         mybir.ActivationFunctionType.Silu,
                                         scale=float(1.0 / S_W))
                    # hsb = silu(g) * (S_U*u) = S_H*h  (S_H == S_U)
                    nc.vector.tensor_mul(hsb[:, i, off:off + n],
                                         hs[:, :n], pu[:, :n])

            # Phase 2: yT[j] = sum_i wd4[j][:, i].T @ h[i]; DR over i-pairs
            for j in range(NJ):
                if j in wdts:
                    wdt = wdts.pop(j)
                else:
                    wdt = wdload.tile([128, NI, 128], F8, tag="wd")
                    nc.sync.dma_start(wdt[:], d_wd4.ap()[j])
                for (off, n) in CH:
                    py = psY.tile([128, 512], F32)
                    for t in range(NI // 2):
                        nc.tensor.matmul(py[:, :n], wdt[:, 2 * t:2 * t + 2, :],
                                         hsb[:, 2 * t:2 * t + 2, off:off + n],
                                         start=(t == 0), stop=(t == NI // 2 - 1),
                                         perf_mode=DR)
                    ob = obp.tile([128, 512], F32)
                    if j % 2 == 0:
                        nc.vector.tensor_copy(ob[:, :n], py[:, :n])
                    else:
                        nc.scalar.copy(ob[:, :n], py[:, :n])
                    nc.sync.dma_start(
                        d_yT.ap()[j * 128:(j + 1) * 128, off:off + n],
                        ob[:, :n])

    nc.compile()
    _nc_cache[key] = nc
    return nc


# --------------------------------------------------------------------------
# Host orchestration
# --------------------------------------------------------------------------

def _rope_tables():
    inv_freq = 1.0 / (10000.0 ** (np.arange(0, HD, 2, dtype=np.float32) / HD))
    t = np.arange(T, dtype=np.float32)
    freqs = np.einsum("i,j->ij", t, inv_freq).astype(np.float32)   # [T, 32]
    emb = np.concatenate([freqs, freqs], axis=-1)                   # [T, 64]
    cos = np.cos(emb).astype(np.float32)
    sin = np.sin(emb).astype(np.float32)
    cosT = np.ascontiguousarray(cos.T)                              # [64, T]
    # stabA pairs with the partition-swapped operand: d<32 -> -sin, d>=32 -> +sin
    sinA = np.empty((HD, T), np.float32)
    sinA[:32] = -sin.T[:32]
    sinA[32:] = sin.T[32:]
    ctab = np.concatenate([cosT] * HPC, axis=0)                     # [128, T]
    stab = np.concatenate([sinA] * HPC, axis=0)
    return ctab, stab


def _causal_masks():
    # additive mask[m, p, f] = 0 where query f sees key (p + 128*m) within
    # the diagonal 512-block, else -1e30 (absorbs s in fp32, exp -> 0).
    f = np.arange(512)[None, :]
    p = np.arange(128)[:, None]
    m4 = np.stack([np.where(f >= p + 128 * m, 0.0, -1e30)
                   for m in range(4)]).astype(np.float32)            # [4,128,512]
    return m4


def _run(nc, in_maps, trace=False, tmpdir=None):
    return run_bass_kernel_spmd(nc, in_maps, list(range(NCORES)),
                                trace=trace, tmpdir=tmpdir)


def kernel(x, norm1_w, norm2_w, qkv_w, proj_w, router_w, wg, wu, wd,
           _trace=False, _stats=None):
    x = np.asarray(x, np.float32)
    B = x.shape[0]
    xf = x.reshape(T, C)

    # ---- host: rms_norm 1 (norm1_w folded into qkv weights) ----
    ms = np.mean(xf * xf, axis=-1, keepdims=True)
    xhat = xf / np.sqrt(ms + EPS)
    xhatT = np.ascontiguousarray(xhat.T)                    # [C, T]

    ctab, stab = _rope_tables()
    masks = _causal_masks()

    qkv_s = (np.asarray(qkv_w, np.float32) * np.asarray(norm1_w, np.float32)[None, :])
    proj = np.asarray(proj_w, np.float32)

    nc_a = build_attention()
    ident = np.eye(128, dtype=np.float32)
    in_maps = []
    for core in range(NCORES):
        h0 = core * HPC
        rows = []
        for g in range(3):  # q, k, v
            rows.append(qkv_s[g * C + h0 * HD: g * C + (h0 + HPC) * HD, :])
        wqkv_c = np.ascontiguousarray(np.concatenate(rows, axis=0).T)  # [C, 384]
        wproj_c = np.ascontiguousarray(proj[:, h0 * HD:(h0 + HPC) * HD].T)  # [128, C]
        in_maps.append({
            "xhatT": xhatT, "wqkv": wqkv_c, "wproj": wproj_c,
            "ctab": ctab, "stab": stab, "mask": masks,
            "identV": ident, "identR": ident,
        })
    res_a = _run(nc_a, in_maps, trace=_trace,
                 tmpdir="/tmp/trace_attn" if _trace else None)
    attn = np.zeros((T, C), np.float32)
    for core in range(NCORES):
        attn += res_a.results[core]["attn_part"]

    xa = xf + attn

    # ---- host: rms_norm 2 + router + top-2 dispatch ----
    ms2 = np.mean(xa * xa, axis=-1, keepdims=True)
    x2 = (xa / np.sqrt(ms2 + EPS)) * np.asarray(norm2_w, np.float32)[None, :]
    logits = x2 @ np.asarray(router_w, np.float32).T        # [T, E]
    topi = np.argsort(-logits, axis=-1)[:, :2]              # [T, 2]
    topv = np.take_along_axis(logits, topi, axis=-1)
    mx = topv.max(axis=-1, keepdims=True)
    ex = np.exp(topv - mx)
    wts = ex / ex.sum(axis=-1, keepdims=True)               # [T, 2]

    idxs, gts = [], []
    for e in range(E):
        sel = np.nonzero((topi == e).any(axis=-1))[0]
        gsel = np.where(topi[sel, 0] == e, wts[sel, 0], wts[sel, 1])
        idxs.append(sel)
        gts.append(gsel.astype(np.float32))
    maxload = max(len(s) for s in idxs)
    cap = max(256, ((maxload + 127) // 128) * 128)

    import ml_dtypes
    F8NP = ml_dtypes.float8_e4m3

    nc_b = build_moe(cap)
    NI, NJ, NKC = HFF // 128, C // 128, C // 128
    in_maps_b = []
    for e in range(E):
        xgT = np.zeros((C, cap), F8NP)
        xgT[:, :len(idxs[e])] = x2[idxs[e]].T.astype(F8NP)
        wg_e = np.asarray(wg[e], np.float32) * S_W
        wu_e = np.asarray(wu[e], np.float32) * S_U
        wd_e = np.asarray(wd[e], np.float32) * S_W
        in_maps_b.append({
            "xgT": xgT,
            "wg4": np.ascontiguousarray(
                wg_e.reshape(NI, 128, NKC, 128).transpose(0, 3, 2, 1)
            ).astype(F8NP),
            "wu4": np.ascontiguousarray(
                wu_e.reshape(NI, 128, NKC, 128).transpose(0, 3, 2, 1)
            ).astype(F8NP),
            "wd4": np.ascontiguousarray(
                wd_e.reshape(NJ, 128, NI, 128).transpose(0, 3, 2, 1)
            ).astype(F8NP),
        })
    res_b = _run(nc_b, in_maps_b, trace=_trace,
                 tmpdir="/tmp/trace_moe" if _trace else None)

    out = xa.copy()
    for e in range(E):
        yT = res_b.results[e]["yT"]                          # [C, cap]
        n = len(idxs[e])
        out[idxs[e]] += yT[:, :n].T * (gts[e] / (S_W * S_H))[:, None]

    if _stats is not None:
        _stats["attn_ns"] = res_a.exec_time_ns
        _stats["moe_ns"] = res_b.exec_time_ns
        _stats["cap"] = cap
    return out.reshape(B, T, C)

